# revision 6
# baseline (speedup 1.0000x reference)
"""GCN (3x GCNConv + mean-pool + LayerNorm + dense + Bayesian head) on 8
Trainium2 NeuronCores.

Strategy: nodes sharded by dst across 8 cores (12500 nodes/core grouped into
98 blocks of 128). All per-layer node tables are [N, 128] bf16 (256B rows for
dma_gather); weights are applied AFTER aggregation (A@(XW) = (A@X)@W), so no
premultiply / transposes are needed: the aggregated block lands in PSUM as
aggT [Fin, 128], is evacuated by the Activation engine, matmul'd with W into
[128, Fout], run through a fused 3-op ELU, and DMA'd straight into the next
layer's AllGather input. Each conv layer gathers source-node rows from a
replicated DRAM table with multi-row `dma_gather` (int16 indices, 4 quadrants
of 25088 rows), aggregating per 128-edge chunk via PSUM-accumulated matmuls
against one-hot selection matrices (GCN norm folded in, built bf16 on
VectorE/GpSimd). Self-loops form a contiguous "diagonal" chunk per block via
direct DMA from core-local data. Tables are group-major so the AllGather can
be issued per 14-block group (overlapping the collective with compute);
pooling = matmul with a one-hot (graph, 1/cnt) matrix, AllReduce, then the
tiny LayerNorm/dense/Bayesian head replicated on every core.
"""
import os, sys
sys.path.insert(0, '/opt/trn_rl_repo')
import numpy as np

from concourse import bass, bacc, tile, mybir
from concourse import bass_utils

# problem dims (hardcoded per spec)
N, E, F0, F1, F2, F3, B = 100000, 640000, 128, 128, 64, 32, 64
C = 8            # cores
P = 128          # partitions
NPC = N // C     # 12500 nodes per core
NB = (NPC + P - 1) // P   # 98 blocks per core
SLOTS = NB * P   # 12544 table rows per core
NTBL = C * SLOTS
NQ = 4           # table quadrants (int16 index reach)
QROWS = NTBL // NQ        # 25088
G_POS = int(os.environ.get('G_POS', '14'))  # blocks per gather/AG group
NAG = int(os.environ.get('NAG', '7'))       # allgathers per layer (1 or NB//G_POS)
GCAP = int(os.environ.get('GCAP', '16'))    # chunks per dma_gather call
SCRATCH = int(os.environ.get('SCRATCH', '65536'))
SHARED = int(os.environ.get('SHARED', '0'))
SPOOL = int(os.environ.get('SPOOL', '3'))   # every SPOOL-th S-build on gpsimd (0=off)

FE = 128                 # stored row width, all tables (256B bf16)
FIN = (F0, F1, F2)       # used row widths per layer
FOUT = (F1, F2, F3)

F32 = mybir.dt.float32
BF16 = mybir.dt.bfloat16
I16 = mybir.dt.int16


def _tblrow(core, pos, lane):
    """Global table row for (core, block-position, lane); group-major when
    NAG>1 so each per-group AllGather output is contiguous."""
    bpg = NB // NAG if NAG > 1 else NB
    gr = bpg * P
    g = pos // bpg
    return g * (C * gr) + core * gr + (pos % bpg) * P + lane


# ---------------------------------------------------------------- host prep
def preprocess(x, edge_index, batch, edge_weight):
    src = np.asarray(edge_index[0], dtype=np.int64)
    dst = np.asarray(edge_index[1], dtype=np.int64)
    ew = np.asarray(edge_weight, dtype=np.float64)
    batch = np.asarray(batch, dtype=np.int64)

    loop = np.arange(N, dtype=np.int64)
    src = np.concatenate([src, loop])
    dst = np.concatenate([dst, loop])
    ew = np.concatenate([ew, np.ones(N)])

    deg = np.bincount(dst, weights=ew, minlength=N)
    dinv = np.where(deg > 0, 1.0 / np.sqrt(np.maximum(deg, 1e-12)), 0.0)
    norm = (dinv[src] * ew * dinv[dst]).astype(np.float32)

    dst_core = dst // NPC
    dst_q = dst % NPC
    dst_b = dst_q // P
    dst_lane = dst_q % P

    # appended self-loops (last N edges) become one direct-DMA "diagonal"
    # chunk per block; regular edges go through dma_gather chunks
    n_reg = len(src) - N

    # per-core block edge counts (regular only) -> block permutation
    # (sorted desc) so the chunk schedule is uniform across cores
    cnts = np.zeros((C, NB), dtype=np.int64)
    np.add.at(cnts, (dst_core[:n_reg], dst_b[:n_reg]), 1)
    order = np.argsort(-cnts, axis=1, kind='stable')          # pos -> block
    posof = np.empty_like(order)                              # block -> pos
    for c in range(C):
        posof[c, order[c]] = np.arange(NB)

    # table row of every global node
    n_all = np.arange(N, dtype=np.int64)
    core_of = n_all // NPC
    q = n_all % NPC
    tblrow = _tblrow(core_of, posof[core_of, q // P], q % P)

    # regular edges keyed by (core, pos, quadrant); sorted by src row
    rc = dst_core[:n_reg]
    epos = posof[rc, dst_b[:n_reg]]
    srow = tblrow[src[:n_reg]]
    equad = srow // QROWS
    sort_idx = np.lexsort((srow, epos, rc))
    s_core = rc[sort_idx]
    s_pos = epos[sort_idx]
    s_quad = equad[sort_idx]

    # counts per (core, pos, quadrant)
    cntq = np.zeros((C, NB, NQ), dtype=np.int64)
    np.add.at(cntq, (s_core, s_pos, s_quad), 1)
    CHQ = ((cntq.max(axis=0) + P - 1) // P).astype(np.int64)  # [NB, NQ]
    CH = 1 + CHQ.sum(axis=1)                                  # incl diag
    cumCH = np.concatenate([[0], np.cumsum(CH)])
    TOTCH = int(cumCH[-1])
    cumq_in_pos = np.concatenate(
        [np.zeros((NB, 1), np.int64), np.cumsum(CHQ, axis=1)], axis=1)
    chq_base = np.concatenate(
        [np.zeros((1, NQ), np.int64), np.cumsum(CHQ, axis=0)], axis=0)  # [NB+1, NQ]
    TOTCHQ = chq_base[-1]                                     # chunks per quad

    # rank of each edge within its (core, pos, quadrant) group
    grp = (s_core * NB + s_pos) * NQ + s_quad
    gcnt = np.zeros(C * NB * NQ, dtype=np.int64)
    np.add.at(gcnt, grp, 1)
    starts = np.concatenate([[0], np.cumsum(gcnt)])[:-1]
    j = np.arange(len(sort_idx)) - starts[grp]
    lane = j % P
    kk = j // P

    # per-chunk S-build scalars (dst lane + norm), col-indexed
    col = cumCH[s_pos] + 1 + cumq_in_pos[s_pos, s_quad] + kk
    dl = np.zeros((C, P, TOTCH), dtype=np.float32)
    nrm = np.zeros((C, P, TOTCH), dtype=np.float32)
    e = sort_idx
    dl[s_core, lane, col] = dst_lane[e].astype(np.float32)
    nrm[s_core, lane, col] = norm[e]

    # int16 index arrays, one per quadrant, in wrap-16 layout:
    # idx j -> [partition j%16 (replicated 8x), column j//16]
    idx16 = []
    jq = (chq_base[s_pos, s_quad] + kk) * P + lane
    lrow = (srow[sort_idx] % QROWS).astype(np.int16)
    for qd in range(NQ):
        cols = int(TOTCHQ[qd]) * P // 16
        arr = np.zeros((C, 16, max(cols, 1)), dtype=np.int16)
        m = s_quad == qd
        arr[s_core[m], jq[m] % 16, jq[m] // 16] = lrow[m]
        idx16.append(np.tile(arr, (1, 8, 1)))                 # replicate

    # pooling: per slot -> graph lane + 1/cnt; diag-chunk scalars
    cnt = np.bincount(batch, minlength=B).astype(np.float64)
    icnt = (1.0 / np.maximum(cnt, 1.0)).astype(np.float32)
    gl = np.zeros((C, P, NB), dtype=np.float32)
    ic = np.zeros((C, P, NB), dtype=np.float32)
    for c in range(C):
        nodes = c * NPC + order[c][:, None] * P + np.arange(P)[None, :]
        valid = (order[c][:, None] * P + np.arange(P)[None, :]) < NPC
        nn = np.where(valid, nodes, 0)
        g = batch[nn]
        gl[c] = np.where(valid, g, 0).T.astype(np.float32)
        ic[c] = np.where(valid, icnt[g], 0.0).T.astype(np.float32)
        dcols = cumCH[:-1]
        dl[c][:, dcols] = np.arange(P, dtype=np.float32)[:, None]
        loop_nrm = np.where(valid, norm[n_reg + nn], 0.0)     # [NB, P]
        nrm[c][:, dcols] = loop_nrm.T.astype(np.float32)

    # x staged in table order (global) + per-core local pos-major copy
    bft = mybir.dt.np(BF16)
    xbf = np.asarray(x, dtype=np.float32).astype(bft)
    xtab = np.zeros((NTBL, F0), dtype=bft)
    xtab[tblrow] = xbf
    xself = np.zeros((C, SLOTS, F0), dtype=bft)
    for c in range(C):
        nodes = c * NPC + order[c][:, None] * P + np.arange(P)[None, :]
        valid = (order[c][:, None] * P + np.arange(P)[None, :]) < NPC
        nn = np.where(valid, nodes, 0)
        rows = np.where(valid[:, :, None], xbf[nn], 0).reshape(SLOTS, F0)
        xself[c] = rows

    return dict(dl=dl, nrm=nrm, gl=gl, ic=ic, xtab=xtab, xself=xself,
                idx16=idx16, CH=CH, CHQ=CHQ, cumCH=cumCH,
                cumq_in_pos=cumq_in_pos, chq_base=chq_base, TOTCHQ=TOTCHQ,
                TOTCH=TOTCH)


# ------------------------------------------------------------- bass builder
def build_bass(pp_data, weights, dt_tab=BF16, reps=1, ablate=()):
    CH = pp_data['CH']; cumCH = pp_data['cumCH']; TOTCH = pp_data['TOTCH']
    CHQ = pp_data['CHQ']; cumq_in_pos = pp_data['cumq_in_pos']
    chq_base = pp_data['chq_base']; TOTCHQ = pp_data['TOTCHQ']

    use_bias = any(np.abs(np.asarray(weights[k])).max() > 0
                   for k in ('b1', 'b2', 'b3'))

    nc = bacc.Bacc("TRN2", target_bir_lowering=False, debug=False,
                   enable_asserts=False, num_devices=C,
                   num_swdge_queues=4, dynamic_dma_scratch_size=SCRATCH)

    def ein(name, shape, dt):
        return nc.dram_tensor(name, shape, dt, kind="ExternalInput").ap()

    xtab_d = ein("xtab", [NTBL, FE], dt_tab)
    xself_d = ein("xself", [SLOTS, FE], dt_tab)
    idx_d = [ein(f"idx{qd}", [P, int(TOTCHQ[qd]) * P // 16], I16)
             for qd in range(NQ)]
    dl_d = ein("dl", [P, TOTCH], F32)
    nrm_d = ein("nrm", [P, TOTCH], F32)
    gl_d = ein("gl", [P, NB], F32)
    ic_d = ein("ic", [P, NB], F32)
    iota_d = ein("iota", [P, P], dt_tab)
    ident_d = ein("ident", [P, P], F32)
    ones_d = ein("ones", [1, P], F32)
    W1_d = ein("W1", [F0, F1], dt_tab)
    W2_d = ein("W2", [F1, F2], dt_tab)
    W3_d = ein("W3", [F2, F3], dt_tab)
    b1_d = ein("b1", [P, F1], F32)
    b2_d = ein("b2", [P, F2], F32)
    b3_d = ein("b3", [P, F3], F32)
    lng_d = ein("lng", [B, F3], F32)
    lnb_d = ein("lnb", [B, F3], F32)
    fcw_d = ein("fcw", [F3, 8], F32)
    fcb_d = ein("fcb", [1, 8], F32)
    wmu_d = ein("wmu", [1, 8], F32)
    wrho_d = ein("wrho", [1, 8], F32)
    weps_d = ein("weps", [1, 8], F32)
    bmu_d = ein("bmu", [1, 1], F32)
    brho_d = ein("brho", [1, 1], F32)
    beps_d = ein("beps", [1, 1], F32)

    out_d = nc.dram_tensor("out", [B, 1], F32, kind="ExternalOutput").ap()

    AF = mybir.ActivationFunctionType
    OP = mybir.AluOpType
    RG = [list(range(C))]
    BPG = NB // NAG if NAG > 1 else NB
    GR = BPG * P

    with tile.TileContext(nc) as tc:
        with tc.tile_pool(name="const", bufs=1) as cp, \
             tc.tile_pool(name="sb", bufs=3) as sb, \
             tc.tile_pool(name="gpool", bufs=int(os.environ.get("GBUFS", "2"))) as gp, \
             tc.tile_pool(name="dpool", bufs=4) as dgp, \
             tc.tile_pool(name="spool", bufs=16) as spl, \
             tc.tile_pool(name="ps_agg", bufs=2, space="PSUM") as ps_agg, \
             tc.tile_pool(name="ps_out", bufs=2, space="PSUM") as ps_out, \
             tc.tile_pool(name="ps_misc", bufs=1, space="PSUM") as ps_misc, \
             tc.tile_pool(name="ps_pool", bufs=1, space="PSUM") as ps_pool, \
             tc.tile_pool(name="dram", bufs=1, space="DRAM") as dp:

            def load_const(ap_d, shape, dt=F32, name=None):
                t = cp.tile(shape, dt, name=name or ap_d.tensor.name + "_sb")
                nc.sync.dma_start(t[:], ap_d)
                return t

            idx_sb = [load_const(idx_d[qd], [P, int(TOTCHQ[qd]) * P // 16],
                                 I16, name=f"idx{qd}_sb") for qd in range(NQ)]
            dl_sb = load_const(dl_d, [P, TOTCH])
            nrm_sb = load_const(nrm_d, [P, TOTCH])
            gl_sb = load_const(gl_d, [P, NB])
            ic_sb = load_const(ic_d, [P, NB])
            iota_sb = load_const(iota_d, [P, P], dt_tab)
            ident_sb = load_const(ident_d, [P, P])
            ones_sb = load_const(ones_d, [1, P])
            W_sb = [load_const(W1_d, [F0, F1], dt_tab),
                    load_const(W2_d, [F1, F2], dt_tab),
                    load_const(W3_d, [F2, F3], dt_tab)]
            b_sb = [load_const(b1_d, [P, F1]), load_const(b2_d, [P, F2]),
                    load_const(b3_d, [P, F3])]
            lng_sb = load_const(lng_d, [B, F3])
            lnb_sb = load_const(lnb_d, [B, F3])
            fcw_sb = load_const(fcw_d, [F3, 8])
            fcb_sb = load_const(fcb_d, [1, 8])
            wmu_sb = load_const(wmu_d, [1, 8])
            wrho_sb = load_const(wrho_d, [1, 8])
            weps_sb = load_const(weps_d, [1, 8])
            bmu_sb = load_const(bmu_d, [1, 1])
            brho_sb = load_const(brho_d, [1, 1])
            beps_sb = load_const(beps_d, [1, 1])

            scnt = [0]

            def build_S(dst_tile, ncol, colidx):
                eng = (nc.gpsimd if SPOOL and scnt[0] % SPOOL == 0
                       else nc.vector)
                scnt[0] += 1
                eng.tensor_scalar(
                    out=dst_tile, in0=iota_sb[:, :ncol],
                    scalar1=dl_sb[:, colidx:colidx + 1],
                    scalar2=nrm_sb[:, colidx:colidx + 1],
                    op0=OP.is_equal, op1=OP.mult)

            for rep in range(reps):
                tabs = [xtab_d]
                cc_in = []
                for nm in ("t2", "t3"):
                    ci = dp.tile([SLOTS, FE], dt_tab, name=f"ccin_{nm}_{rep}")
                    to = dp.tile([NTBL, FE], dt_tab, name=f"tab_{nm}_{rep}",
                                 addr_space="Shared" if (NAG == 1 and SHARED)
                                 else "Local")
                    cc_in.append(ci)
                    tabs.append(to)

                pool_ps = ps_pool.tile([B, F3], F32, tag="pp", name="pool_ps")

                qrr = [0]
                for L in range(3):
                    Fin = FIN[L]
                    Fout = FOUT[L]
                    tab = tabs[L]
                    for g0 in range(0, NB, G_POS):
                        poss = range(g0, min(g0 + G_POS, NB))
                        # one multi-row gather per quadrant for this group
                        Gq = []
                        for qd in range(NQ):
                            nch = int(CHQ[list(poss), qd].sum())
                            ch0 = int(chq_base[g0, qd])
                            if nch == 0:
                                Gq.append(None)
                                continue
                            Gt = gp.tile([P, nch * FE], dt_tab,
                                         tag=f"G{qd}", name=f"G{qd}")
                            if 'nogather' not in ablate:
                                for s0 in range(0, nch, GCAP):
                                    sn = min(GCAP, nch - s0)
                                    nc.gpsimd.dma_gather(
                                        out_ap=Gt[:, s0 * FE:(s0 + sn) * FE]
                                        .rearrange("p (c e) -> p c e", e=FE),
                                        in_ap=tab[qd * QROWS:(qd + 1) * QROWS, :],
                                        idxs_ap=idx_sb[qd][
                                            :, (ch0 + s0) * 8:(ch0 + s0 + sn) * 8],
                                        num_idxs=sn * P,
                                        num_idxs_reg=sn * P,
                                        elem_size=FE,
                                        queue_num=qrr[0] % 4,
                                    )
                                    qrr[0] += 1
                            Gq.append(Gt)
                        for pos in poss:
                            c0 = int(cumCH[pos])
                            aggT = ps_agg.tile([Fin, P], F32, tag="aggT",
                                               name="aggT")
                            # diagonal (self-loop) chunk via direct DMA
                            Gd = dgp.tile([P, Fin], dt_tab, tag="Gd",
                                          name="Gd")
                            src_loc = xself_d if L == 0 else cc_in[L - 1]
                            nc.sync.dma_start(
                                Gd[:], src_loc[pos * P:(pos + 1) * P, :Fin])
                            S_b = spl.tile([P, P], dt_tab, tag="S",
                                           name="S_b")
                            build_S(S_b[:], P, c0)
                            nlast = int(CH[pos]) - 1
                            nc.tensor.matmul(out=aggT[:], lhsT=Gd[:],
                                             rhs=S_b[:], start=True,
                                             stop=(nlast == 0))
                            ki = 0
                            for qd in range(NQ):
                                nch_q = int(CHQ[pos, qd])
                                off = int(CHQ[list(poss), qd][:pos - g0].sum())
                                for k in range(nch_q):
                                    ki += 1
                                    S_b = spl.tile([P, P], dt_tab, tag="S",
                                                   name="S_b")
                                    build_S(S_b[:], P, c0 + 1 +
                                            int(cumq_in_pos[pos, qd]) + k)
                                    gsl = Gq[qd][:, (off + k) * FE:
                                                 (off + k) * FE + Fin]
                                    nc.tensor.matmul(
                                        out=aggT[:], lhsT=gsl, rhs=S_b[:],
                                        start=False, stop=(ki == nlast))
                            # evacuate via Activation engine, apply weights
                            aggs = sb.tile([Fin, P], dt_tab, tag="aggs",
                                           name="aggs")
                            nc.scalar.activation(aggs[:], aggT[:], AF.Copy)
                            hw = ps_out.tile([P, Fout], F32, tag="ops",
                                             name="hw")
                            nc.tensor.matmul(out=hw[:], lhsT=aggs[:],
                                             rhs=W_sb[L][:], start=True,
                                             stop=True)
                            # fused ELU: h = max(exp(min(x,0)) - 1, x)
                            h = sb.tile([P, Fout], dt_tab, tag="h", name="h")
                            t1 = sb.tile([P, Fout], dt_tab, tag="elu1",
                                         name="t1")
                            t2 = sb.tile([P, Fout], dt_tab, tag="elu2",
                                         name="t2")
                            if use_bias:
                                xb = sb.tile([P, Fout], F32, tag="xb",
                                             name="xb")
                                nc.vector.tensor_tensor(
                                    xb[:], hw[:], b_sb[L][:, :Fout],
                                    op=OP.add)
                                nc.vector.tensor_scalar_min(t1[:], xb[:], 0.0)
                                nc.scalar.activation(t2[:], t1[:], AF.Exp)
                                nc.vector.scalar_tensor_tensor(
                                    out=h[:], in0=t2[:], scalar=-1.0,
                                    in1=xb[:], op0=OP.add, op1=OP.max)
                            else:
                                nc.vector.tensor_scalar_min(t1[:], hw[:], 0.0)
                                nc.scalar.activation(t2[:], t1[:], AF.Exp)
                                nc.vector.scalar_tensor_tensor(
                                    out=h[:], in0=t2[:], scalar=-1.0,
                                    in1=hw[:], op0=OP.add, op1=OP.max)
                            if L < 2:
                                nc.sync.dma_start(
                                    cc_in[L][pos * P:(pos + 1) * P, :Fout],
                                    h[:])
                            else:
                                Sp = spl.tile([P, B], dt_tab, tag="Sp",
                                              name="Sp")
                                nc.vector.tensor_scalar(
                                    out=Sp[:], in0=iota_sb[:, :B],
                                    scalar1=gl_sb[:, pos:pos + 1],
                                    scalar2=ic_sb[:, pos:pos + 1],
                                    op0=OP.is_equal, op1=OP.mult)
                                nc.tensor.matmul(out=pool_ps[:], lhsT=Sp[:],
                                                 rhs=h[:], start=(pos == 0),
                                                 stop=(pos == NB - 1),
                                                 skip_group_check=True)
                        if L < 2 and NAG > 1 and 'nocc' not in ablate:
                            gi = g0 // BPG
                            nc.gpsimd.collective_compute(
                                "AllGather", OP.bypass, replica_groups=RG,
                                ins=[cc_in[L][gi * GR:(gi + 1) * GR, :].opt()],
                                outs=[tabs[L + 1][gi * C * GR:
                                                  (gi + 1) * C * GR, :].opt()])
                    if L < 2 and NAG == 1 and 'nocc' not in ablate:
                        nc.gpsimd.collective_compute(
                            "AllGather", OP.bypass, replica_groups=RG,
                            ins=[cc_in[L].opt()], outs=[tabs[L + 1].opt()])

                # ---- pooled mean allreduce + head
                pool_sb = sb.tile([B, F3], F32, tag="pool_sb", name="pool_sb")
                nc.vector.tensor_copy(pool_sb[:], pool_ps[:])
                ar_in = dp.tile([B, F3], F32, name=f"arin_{rep}")
                ar_out = dp.tile([B, F3], F32, name=f"arout_{rep}")
                nc.sync.dma_start(ar_in[:], pool_sb[:])
                gm = sb.tile([B, F3], F32, tag="gm", name="gm")
                if 'nocc' not in ablate:
                    nc.gpsimd.collective_compute(
                        "AllReduce", OP.add, replica_groups=RG,
                        ins=[ar_in.opt()], outs=[ar_out.opt()])
                    nc.sync.dma_start(gm[:], ar_out[:])
                else:
                    nc.sync.dma_start(gm[:], ar_in[:])

                # LayerNorm over 32 features
                mu = sb.tile([B, 1], F32, tag="mu", name="mu")
                nc.vector.reduce_sum(out=mu[:], in_=gm[:],
                                     axis=mybir.AxisListType.X)
                nc.vector.tensor_scalar_mul(mu[:], mu[:], 1.0 / F3)
                xc = sb.tile([B, F3], F32, tag="xc", name="xc")
                nc.vector.tensor_scalar(out=xc[:], in0=gm[:], scalar1=mu[:],
                                        scalar2=None, op0=OP.subtract)
                sq = sb.tile([B, F3], F32, tag="sq", name="sq")
                nc.scalar.activation(sq[:], xc[:], AF.Square)
                vv = sb.tile([B, 1], F32, tag="vv", name="vv")
                nc.vector.reduce_sum(out=vv[:], in_=sq[:],
                                     axis=mybir.AxisListType.X)
                nc.vector.tensor_scalar_mul(vv[:], vv[:], 1.0 / F3)
                nc.vector.tensor_scalar_add(vv[:], vv[:], 1e-5)
                sd = sb.tile([B, 1], F32, tag="sd", name="sd")
                nc.scalar.activation(sd[:], vv[:], AF.Sqrt)
                rs = sb.tile([B, 1], F32, tag="rs", name="rs")
                nc.vector.reciprocal(rs[:], sd[:])
                nc.vector.tensor_scalar_mul(xc[:], xc[:], rs[:])
                y = sb.tile([B, F3], F32, tag="y", name="y")
                nc.vector.tensor_tensor(y[:], xc[:], lng_sb[:], op=OP.mult)
                nc.vector.tensor_tensor(y[:], y[:], lnb_sb[:], op=OP.add)

                # h2 = elu(y @ fc_w + fc_b)
                yT_ps = ps_misc.tile([F3, B], F32, tag="trp", name="yT_ps")
                nc.tensor.transpose(out=yT_ps[:], in_=y[:],
                                    identity=ident_sb[:B, :B])
                yT = sb.tile([F3, B], F32, tag="yTs", name="yT")
                nc.vector.tensor_copy(yT[:], yT_ps[:])
                h2_ps = ps_out.tile([B, 8], F32, tag="ops", name="h2_ps")
                nc.tensor.matmul(out=h2_ps[:], lhsT=yT[:], rhs=fcw_sb[:],
                                 start=True, stop=False)
                nc.tensor.matmul(out=h2_ps[:], lhsT=ones_sb[:1, :B],
                                 rhs=fcb_sb[:], start=False, stop=True)
                h2 = sb.tile([B, 8], F32, tag="h2s", name="h2")
                t1h = sb.tile([B, 8], F32, tag="ht1", name="ht1")
                nc.vector.tensor_scalar_min(t1h[:], h2_ps[:], 0.0)
                t2h = sb.tile([B, 8], F32, tag="ht2", name="ht2")
                nc.scalar.activation(t2h[:], t1h[:], AF.Exp)
                nc.vector.tensor_scalar_add(t2h[:], t2h[:], -1.0)
                nc.vector.tensor_tensor(h2[:], h2_ps[:], t2h[:], op=OP.max)

                # bayes weights: w = mu + softplus(rho) * eps
                weff = sb.tile([1, 8], F32, tag="weff", name="weff")
                nc.scalar.activation(weff[:], wrho_sb[:], AF.Exp)
                nc.vector.tensor_scalar_add(weff[:], weff[:], 1.0)
                nc.scalar.activation(weff[:], weff[:], AF.Ln)
                nc.vector.tensor_tensor(weff[:], weff[:], weps_sb[:],
                                        op=OP.mult)
                nc.vector.tensor_tensor(weff[:], weff[:], wmu_sb[:],
                                        op=OP.add)
                beff = sb.tile([1, 1], F32, tag="beff", name="beff")
                nc.scalar.activation(beff[:], brho_sb[:], AF.Exp)
                nc.vector.tensor_scalar_add(beff[:], beff[:], 1.0)
                nc.scalar.activation(beff[:], beff[:], AF.Ln)
                nc.vector.tensor_tensor(beff[:], beff[:], beps_sb[:],
                                        op=OP.mult)
                nc.vector.tensor_tensor(beff[:], beff[:], bmu_sb[:],
                                        op=OP.add)
                wb_ps = ps_misc.tile([B, 8], F32, tag="pm", name="wb_ps")
                nc.tensor.matmul(out=wb_ps[:], lhsT=ones_sb[:1, :B],
                                 rhs=weff[:], start=True, stop=True)
                bb_ps = ps_misc.tile([B, 1], F32, tag="pm", name="bb_ps")
                nc.tensor.matmul(out=bb_ps[:], lhsT=ones_sb[:1, :B],
                                 rhs=beff[:], start=True, stop=True)
                prod = sb.tile([B, 8], F32, tag="prod", name="prod")
                nc.vector.tensor_tensor(prod[:], h2[:], wb_ps[:], op=OP.mult)
                red = sb.tile([B, 1], F32, tag="red", name="red")
                nc.vector.reduce_sum(out=red[:], in_=prod[:],
                                     axis=mybir.AxisListType.X)
                res = sb.tile([B, 1], F32, tag="res", name="res")
                nc.vector.tensor_tensor(res[:], red[:], bb_ps[:], op=OP.add)
                nc.sync.dma_start(out_d, res[:])

    nc.compile()

    bft = mybir.dt.np(BF16)
    common = dict(
        xtab=pp_data['xtab'],
        iota=np.tile(np.arange(P, dtype=np.float32), (P, 1)).astype(bft),
        ident=np.eye(P, dtype=np.float32),
        ones=np.ones((1, P), dtype=np.float32),
        W1=np.asarray(weights['W1'], np.float32).astype(bft),
        W2=np.asarray(weights['W2'], np.float32).astype(bft),
        W3=np.asarray(weights['W3'], np.float32).astype(bft),
        b1=np.tile(weights['b1'][None, :], (P, 1)).astype(np.float32),
        b2=np.tile(weights['b2'][None, :], (P, 1)).astype(np.float32),
        b3=np.tile(weights['b3'][None, :], (P, 1)).astype(np.float32),
        lng=np.tile(weights['ln_g'][None, :], (B, 1)).astype(np.float32),
        lnb=np.tile(weights['ln_b'][None, :], (B, 1)).astype(np.float32),
        fcw=np.asarray(weights['fc_w'], np.float32),
        fcb=np.asarray(weights['fc_b'], np.float32)[None, :],
        wmu=np.asarray(weights['w_mu'], np.float32),
        wrho=np.asarray(weights['w_rho'], np.float32),
        weps=np.asarray(weights['w_eps'], np.float32),
        bmu=np.asarray(weights['b_mu'], np.float32)[None, :],
        brho=np.asarray(weights['b_rho'], np.float32)[None, :],
        beps=np.asarray(weights['b_eps'], np.float32)[None, :],
    )
    return nc, common


def make_in_maps(pp_data, common):
    in_maps = []
    for c in range(C):
        m = dict(common)
        m['xself'] = pp_data['xself'][c]
        for qd in range(NQ):
            m[f'idx{qd}'] = pp_data['idx16'][qd][c]
        m['dl'] = pp_data['dl'][c]
        m['nrm'] = pp_data['nrm'][c]
        m['gl'] = pp_data['gl'][c]
        m['ic'] = pp_data['ic'][c]
        in_maps.append(m)
    return in_maps


DT_TAB = BF16


def kernel(**inputs):
    x = np.asarray(inputs['x'])
    pp = preprocess(x, inputs['edge_index'], inputs['batch'],
                    inputs['edge_weight'])
    weights = {k: np.asarray(v) for k, v in inputs.items()
               if k not in ('x', 'edge_index', 'batch', 'edge_weight')}
    nc, common = build_bass(pp, weights, dt_tab=DT_TAB, reps=1)
    in_maps = make_in_maps(pp, common)
    res = bass_utils.run_bass_kernel_spmd(nc, in_maps, core_ids=list(range(C)))
    return res.results[0]['out'].astype(np.float32)
